# revision 1
# baseline (speedup 1.0000x reference)
"""MPI compositing + homography warp kernel for Trainium2 (8 NeuronCores), v2.

Sharding: core = (s, ph) = 4 sources x 2 plane-halves. Each core runs the
full 32-plane over-compositing scan on the full image (redundant across the
ph pair -- cheaper than a collective), then warps its own 16 planes x 10
channels with transpose-free bf16 matmul chains.

Transpose-free warp of F = Wy @ S @ Wx^T per channel:
  U^T[xi, yo] = sum_yi S[yi, xi] WyT[yi, yo]   (lhsT = S chunk, rhs = wyt)
  F[yo, xo]   = sum_xi U^T[xi, yo] WxT[xi, xo] (lhsT = U^T chunk, rhs = wxt)
PSUM tiles are single full banks [128, 512] f32 so each PSUM->SBUF
evacuation is one big copy (per-op overhead ~300ns dominates small copies).

SPMD acc trick: acc channel of plane d needs overs[d-2]; the snapshot index
differs between the two ph cores but the program must be uniform. The acc
mm1' therefore accumulates BOTH candidate snapshots (S[max(do-2,0)] and
S[do+14]) with per-core weight data -- the unneeded candidate's Wy is all
zeros on that core.

T channel is host-precomputed (exact f64 suffix product -> bf16 input),
removing the backward scan and its ordering constraints entirely.
"""

import os
import sys

import numpy as np

sys.path.insert(0, "/opt/trn_rl_repo")

P, S, H, W = 32, 4, 256, 256
POWN = P // 2
NCORES = 8
NCH = 10

PMCA_DT = os.environ.get("KERNEL_PMCA_DT", "bf16")  # "bf16" or "f32"


def _compute_sample_coords(mpi_planes, pose_tgt, intrins_src, intrins_tgt):
    """Exact reference math for sample coords, float64. -> ix, iy (P,S,H,W)."""
    Kinv = np.linalg.inv(intrins_tgt.astype(np.float64))
    gx, gy = np.meshgrid(
        np.arange(W, dtype=np.float64), np.arange(H, dtype=np.float64)
    )
    pix = np.stack([gx.ravel(), gy.ravel(), np.ones(H * W)])  # (3, HW)
    cam_dir = Kinv @ pix  # (3, HW)
    ix = np.empty((P, S, H, W))
    iy = np.empty((P, S, H, W))
    for s in range(S):
        K4 = np.zeros((4, 4))
        K4[:3, :3] = intrins_src[s].astype(np.float64)
        K4[3, 3] = 1.0
        proj = K4 @ pose_tgt[s].astype(np.float64)
        for p in range(P):
            cam = np.concatenate(
                [cam_dir * np.float64(mpi_planes[p]), np.ones((1, H * W))], 0
            )
            upc = proj @ cam
            z = upc[2] + 1e-10
            ix[p, s] = (upc[0] / z).reshape(H, W)
            iy[p, s] = (upc[1] / z).reshape(H, W)
    return ix, iy


def _bilinear_matrix(coord_1d, n_in):
    """1D resample matrix M[out, in] with reference tap/validity semantics."""
    n_out = coord_1d.shape[0]
    M = np.zeros((n_out, n_in), np.float64)
    c0 = np.floor(coord_1d)
    w1 = coord_1d - c0
    w0 = 1.0 - w1
    i0 = c0.astype(np.int64)
    r = np.arange(n_out)
    v0 = (i0 >= 0) & (i0 <= n_in - 1)
    np.add.at(M, (r[v0], i0[v0]), w0[v0])
    i1 = i0 + 1
    v1 = (i1 >= 0) & (i1 <= n_in - 1)
    np.add.at(M, (r[v1], i1[v1]), w1[v1])
    return M


def _reference_numpy(colors, alphas, imgs_src, mpi_planes, pose_tgt,
                     intrins_src, intrins_tgt):
    """Pure-numpy replica of the reference (generic fallback)."""
    Pn, Sn, Hh, Ww = alphas.shape
    ca = 1.0 - alphas
    pm = colors * alphas[..., None]
    overs = np.empty_like(pm)
    over = np.zeros_like(pm[0])
    for d in range(Pn):
        over = over * ca[d][..., None] + pm[d]
        overs[d] = over
    acc = overs[np.maximum(np.arange(Pn) - 2, 0)]
    bro = np.broadcast_to(overs[-1][None], (Pn, Sn, Hh, Ww, 3))
    rc = np.cumprod(ca[::-1], axis=0)[::-1]
    T = np.concatenate([rc[1:], np.ones_like(rc[:1])], axis=0)
    src = np.broadcast_to(imgs_src[None], (Pn, Sn, Hh, Ww, 3))
    stacked = np.concatenate([T[..., None], acc, bro, src], axis=-1)

    ix, iy = _compute_sample_coords(mpi_planes, pose_tgt, intrins_src,
                                    intrins_tgt)
    out = np.empty((Pn, Sn, NCH, Hh, Ww), np.float32)
    for p in range(Pn):
        for s in range(Sn):
            img = stacked[p, s]
            x0 = np.floor(ix[p, s])
            y0 = np.floor(iy[p, s])
            wx1 = ix[p, s] - x0
            wx0 = 1.0 - wx1
            wy1 = iy[p, s] - y0
            wy0 = 1.0 - wy1

            def gather(xx, yy):
                valid = (xx >= 0) & (xx <= Ww - 1) & (yy >= 0) & (yy <= Hh - 1)
                xc = np.clip(xx, 0, Ww - 1).astype(np.int64)
                yc = np.clip(yy, 0, Hh - 1).astype(np.int64)
                return img[yc, xc] * valid[..., None]

            warped = (gather(x0, y0) * (wx0 * wy0)[..., None]
                      + gather(x0 + 1, y0) * (wx1 * wy0)[..., None]
                      + gather(x0, y0 + 1) * (wx0 * wy1)[..., None]
                      + gather(x0 + 1, y0 + 1) * (wx1 * wy1)[..., None])
            out[p, s] = warped.transpose(2, 0, 1).astype(np.float32)
    return out


_CACHED = {}


def _build_bass_program():
    """Build (once) the SPMD Bass program shared by all 8 cores."""
    if "nc" in _CACHED:
        return _CACHED["nc"]

    import concourse.bacc as bacc
    import concourse.mybir as mybir
    from concourse import tile

    f32 = mybir.dt.float32
    bf16 = mybir.dt.bfloat16
    pmca_dt = bf16 if PMCA_DT == "bf16" else f32

    nc = bacc.Bacc(
        "TRN2", target_bir_lowering=False, debug=False,
        enable_asserts=False, num_devices=NCORES,
    )

    # HBM tensors (per-core data via in_maps)
    pm_d = nc.dram_tensor("pm", [P // 2, 128, 2, 3, 512], f32,
                          kind="ExternalInput").ap()
    ca_d = nc.dram_tensor("ca", [P // 2, 128, 2, 512], bf16,
                          kind="ExternalInput").ap()
    wgt1_d = nc.dram_tensor("wgt1", [POWN, 128, 2, 768], bf16,
                            kind="ExternalInput").ap()
    wgt2_d = nc.dram_tensor("wgt2", [POWN, 128, 2, 1024], bf16,
                            kind="ExternalInput").ap()
    src_d = nc.dram_tensor("src", [128, 3, 2, 256], bf16,
                           kind="ExternalInput").ap()
    out1_d = nc.dram_tensor("out1", [POWN, 128, 4, 512], bf16,
                            kind="ExternalOutput").ap()
    out2a_d = nc.dram_tensor("out2a", [POWN, 128, 3, 512], bf16,
                             kind="ExternalOutput").ap()
    out2b_d = nc.dram_tensor("out2b", [POWN, 128, 3, 512], bf16,
                             kind="ExternalOutput").ap()

    with tile.TileContext(nc) as tc:
        # copy-engine rotation: DVE and Scalar are ~1.4x faster per col
        # than Pool, so Pool gets 1 in 5 copies.
        def _v_copy(out, in_):
            nc.vector.tensor_copy(out, in_)

        def _s_copy(out, in_):
            nc.scalar.copy(out, in_)

        def _g_copy(out, in_):
            nc.gpsimd.tensor_copy(out, in_)

        # PSUM evacuations can only run on DVE/Scalar (GPSIMD cannot read
        # PSUM); DVE also carries the scan chain, so Scalar gets 2 of 3.
        copy_cycle = [_s_copy, _v_copy, _s_copy]
        ci_ctr = [0]

        def copy_eng():
            e = copy_cycle[ci_ctr[0] % len(copy_cycle)]
            ci_ctr[0] += 1
            return e

        with (
            tc.tile_pool(name="srcp", bufs=1) as srcp,
            tc.tile_pool(name="pmca", bufs=2) as pmca_pool,
            tc.tile_pool(name="state", bufs=2) as state_pool,
            tc.tile_pool(name="snap", bufs=20) as snap_pool,
            tc.tile_pool(name="wgt1", bufs=2) as wgt1_pool,
            tc.tile_pool(name="wgt2", bufs=2) as wgt2_pool,
            tc.tile_pool(name="wgt3", bufs=2) as wgt3_pool,
            tc.tile_pool(name="ut", bufs=6) as ut_pool,
            tc.tile_pool(name="fpk4", bufs=3) as fpk4_pool,
            tc.tile_pool(name="fpk3", bufs=4) as fpk3_pool,
            tc.tile_pool(name="psum_ut", bufs=4, space="PSUM") as psum_ut,
            tc.tile_pool(name="psum_f", bufs=4, space="PSUM") as psum_f,
        ):
            src_sb = srcp.tile([128, 3, 2, 256], bf16, name="src_sb")
            nc.sync.dma_start(src_sb[:], src_d[:])

            def warp(s_aps_wyts, wxt_ap, fpk_t, slot):
                """One channel image: mm1' (possibly 2-source blend),
                evacuate U^T, mm2, evacuate F into fpk slot."""
                ut_ps = psum_ut.tile([128, 512], f32, tag="utps", name="ut_ps")
                nsrc = len(s_aps_wyts)
                for m in range(2):
                    idx = 0
                    for s_ap, wyt_ap in s_aps_wyts:
                        for k in range(2):
                            nc.tensor.matmul(
                                ut_ps[:, m * 256:(m + 1) * 256],
                                s_ap[:, k, m * 128:(m + 1) * 128],
                                wyt_ap[:, k, :],
                                start=(idx == 0), stop=(idx == 2 * nsrc - 1),
                            )
                            idx += 1
                ut_sb = ut_pool.tile([128, 2, 256], bf16, tag="ut",
                                     name="ut_sb")
                copy_eng()(ut_sb[:].rearrange("p a b -> p (a b)"), ut_ps[:])
                f_ps = psum_f.tile([128, 512], f32, tag="fps", name="f_ps")
                for h in range(2):
                    for k in range(2):
                        nc.tensor.matmul(
                            f_ps[:, h * 256:(h + 1) * 256],
                            ut_sb[:, k, h * 128:(h + 1) * 128],
                            wxt_ap[:, k, :],
                            start=(k == 0), stop=(k == 1),
                        )
                copy_eng()(fpk_t[:, slot, :], f_ps[:])

            # ---- main loop: scan 2 planes/iter, fused with T/src warps
            # and (from iter 7) acc warps. Fused emission keeps each
            # engine's in-order queue interleaved so the PE never waits
            # ~80us for copies queued behind the whole scan.
            wg1 = [None, None]
            for j in range(2):
                wg1[j] = wgt1_pool.tile([128, 2, 768], bf16, tag="wgt1",
                                        name="wg1")
                nc.sync.dma_start(wg1[j][:], wgt1_d[j])
            wg2 = [None, None]
            for j in range(2):
                wg2[j] = wgt2_pool.tile([128, 2, 1024], bf16, tag="wgt2",
                                        name="wg2")
                nc.sync.dma_start(wg2[j][:], wgt2_d[j])

            # two parallel scan chains: ch0+ch1 fused on DVE, ch2 on Pool
            st01 = [None]             # f32 state [128, 2, 512]
            st2 = [None]              # f32 state [128, 512]
            snaps = [None] * P        # per-plane bf16 snapshots [128,3,2,256]

            def do_acc_warp(do):
                wg = wg2[do % 2]
                fpk = fpk3_pool.tile([128, 3, 512], bf16, tag="fpk3",
                                     name="fpk3")
                wxt = wg[:, :, 256:512]
                wytA = wg[:, :, 512:768]
                wytB = wg[:, :, 768:1024]
                snapA = snaps[max(do - 2, 0)]
                snapB = snaps[do + POWN - 2]
                for c in range(3):
                    warp([(snapA[:, c], wytA), (snapB[:, c], wytB)],
                         wxt, fpk, c)
                nc.sync.dma_start(
                    out2a_d[do], fpk[:].rearrange("p c x -> p (c x)"))
                if do + 2 < POWN:
                    wg2[do % 2] = wgt2_pool.tile([128, 2, 1024], bf16,
                                                 tag="wgt2", name="wg2")
                    nc.sync.dma_start(wg2[do % 2][:], wgt2_d[do + 2])

            for tp in range(P // 2):
                pcm = pmca_pool.tile([128, 2, 3, 512], f32, tag="pm",
                                     name="pcm")
                nc.sync.dma_start(pcm[:], pm_d[tp])
                pca = pmca_pool.tile([128, 2, 512], bf16, tag="ca",
                                     name="pca")
                nc.sync.dma_start(pca[:], ca_d[tp])
                for tt in range(2):
                    t = 2 * tp + tt
                    n01 = state_pool.tile([128, 2, 512], f32, tag="st01",
                                          name="st01")
                    n2 = state_pool.tile([128, 512], f32, tag="st2",
                                         name="st2")
                    if t == 0:
                        nc.vector.tensor_copy(n01[:], pcm[:, tt, 0:2, :])
                        nc.gpsimd.tensor_copy(n2[:], pcm[:, tt, 2, :])
                    else:
                        t01 = state_pool.tile([128, 2, 512], f32, tag="tmp01",
                                              name="tmp01")
                        t2 = state_pool.tile([128, 512], f32, tag="tmp2",
                                             name="tmp2")
                        ca_b = pca[:, tt:tt + 1, :].broadcast_to([128, 2, 512])
                        nc.vector.tensor_mul(t01[:], st01[0][:], ca_b)
                        nc.vector.tensor_add(n01[:], t01[:], pcm[:, tt, 0:2, :])
                        nc.gpsimd.tensor_mul(t2[:], st2[0][:], pca[:, tt, :])
                        nc.gpsimd.tensor_add(n2[:], t2[:], pcm[:, tt, 2, :])
                    st01[0] = n01
                    st2[0] = n2
                    if t != P - 2:  # S[30] is never read
                        sn = snap_pool.tile([128, 3, 2, 256], bf16,
                                            tag="snap", name="snap")
                        nc.scalar.copy(
                            sn[:, 0:2].rearrange("p c a b -> p (c a b)"),
                            n01[:].rearrange("p c x -> p (c x)"))
                        nc.scalar.copy(
                            sn[:, 2].rearrange("p a b -> p (a b)"), n2[:])
                        snaps[t] = sn

                # T + src warps for own-plane index do = tp
                do = tp
                wg = wg1[do % 2]
                fpk = fpk4_pool.tile([128, 4, 512], bf16, tag="fpk4",
                                     name="fpk4")
                wyt = wg[:, :, 0:256]
                wxt = wg[:, :, 256:512]
                tch = wg[:, :, 512:768]
                warp([(tch, wyt)], wxt, fpk, 0)
                for c in range(3):
                    warp([(src_sb[:, c], wyt)], wxt, fpk, 1 + c)
                nc.sync.dma_start(
                    out1_d[do], fpk[:].rearrange("p c x -> p (c x)"))
                if do + 2 < POWN:
                    wg1[do % 2] = wgt1_pool.tile([128, 2, 768], bf16,
                                                 tag="wgt1", name="wg1")
                    nc.sync.dma_start(wg1[do % 2][:], wgt1_d[do + 2])

                # acc warps whose snapB (= overs[do'+14]) just became ready
                for dacc in (2 * tp - 14, 2 * tp - 13):
                    if 0 <= dacc < POWN:
                        do_acc_warp(dacc)

            # ---- phase 2b: bro warps ----------------------------------
            wg3 = [None, None]
            for j in range(2):
                wg3[j] = wgt3_pool.tile([128, 2, 512], bf16, tag="wgt3",
                                        name="wg3")
                nc.sync.dma_start(wg3[j][:], wgt1_d[j][:, :, 0:512])
            for do in range(POWN):
                wg = wg3[do % 2]
                fpk = fpk3_pool.tile([128, 3, 512], bf16, tag="fpk3",
                                     name="fpk3")
                wyt = wg[:, :, 0:256]
                wxt = wg[:, :, 256:512]
                for c in range(3):
                    warp([(snaps[P - 1][:, c], wyt)], wxt, fpk, c)
                nc.sync.dma_start(
                    out2b_d[do], fpk[:].rearrange("p c x -> p (c x)"))
                if do + 2 < POWN:
                    wg3[do % 2] = wgt3_pool.tile([128, 2, 512], bf16,
                                                 tag="wgt3", name="wg3")
                    nc.sync.dma_start(wg3[do % 2][:],
                                      wgt1_d[do + 2][:, :, 0:512])

    nc.compile()
    _CACHED["nc"] = nc
    return nc


def _host_prepare(colors, alphas, imgs_src, mpi_planes, pose_tgt,
                  intrins_src, intrins_tgt):
    """Build per-core input maps. Returns (in_maps, separable)."""
    import ml_dtypes

    bfh = ml_dtypes.bfloat16
    pmca_np = bfh if PMCA_DT == "bf16" else np.float32

    ix, iy = _compute_sample_coords(mpi_planes, pose_tgt, intrins_src,
                                    intrins_tgt)
    dev_x = np.abs(ix - ix[:, :, :1, :]).max()
    dev_y = np.abs(iy - iy[:, :, :, :1]).max()
    if dev_x > 1e-3 or dev_y > 1e-3:
        return None, False

    ix1 = ix[:, :, 0, :]  # (P, S, W)
    iy1 = iy[:, :, :, 0]  # (P, S, H)

    ca64 = 1.0 - alphas.astype(np.float64)          # (P,S,H,W)
    pm32 = (colors.astype(np.float32)
            * alphas.astype(np.float32)[..., None])  # (P,S,H,W,3)
    rc = np.cumprod(ca64[::-1], axis=0)[::-1]
    T64 = np.concatenate([rc[1:], np.ones_like(rc[:1])], axis=0)

    in_maps = []
    for core in range(NCORES):
        s, ph = divmod(core, 2)
        # pm: [P//2, 128, 2, 3, 512] f32; ca: [P//2, 128, 2, 512] bf16;
        # [tp, p, tt, (c,) c2*256+x] = value of plane 2tp+tt at image row
        # 128*c2+p, col x
        pmv = pm32[:, s].reshape(P, 2, 128, 256, 3)     # (P,c2,p,x,3)
        cav = ca64[:, s].astype(np.float32).reshape(P, 2, 128, 256)
        pm = (pmv.transpose(0, 2, 4, 1, 3)              # (P,p,3,c2,x)
              .reshape(P // 2, 2, 128, 3, 512).transpose(0, 2, 1, 3, 4))
        ca = (cav.transpose(0, 2, 1, 3)                 # (P,p,c2,x)
              .reshape(P // 2, 2, 128, 512).transpose(0, 2, 1, 3))

        wgt1 = np.zeros((POWN, 128, 2, 768), np.float32)
        wgt2 = np.zeros((POWN, 128, 2, 1024), np.float32)
        Tv = T64[:, s].astype(np.float32).reshape(P, 2, 128, 256)
        for do in range(POWN):
            d = ph * POWN + do
            My = _bilinear_matrix(iy1[d, s], H)   # (H out, H in)
            Mx = _bilinear_matrix(ix1[d, s], W)   # (W out, W in)
            wyt = My.T.astype(np.float32).reshape(2, 128, 256)  # [k,p,yo]
            wxt = Mx.T.astype(np.float32).reshape(2, 128, 256)  # [k,p,xo]
            wgt1[do, :, :, 0:256] = wyt.transpose(1, 0, 2)
            wgt1[do, :, :, 256:512] = wxt.transpose(1, 0, 2)
            wgt1[do, :, :, 512:768] = Tv[d].transpose(1, 0, 2)
            wgt2[do, :, :, 0:256] = wyt.transpose(1, 0, 2)
            wgt2[do, :, :, 256:512] = wxt.transpose(1, 0, 2)
            if ph == 0:
                wgt2[do, :, :, 512:768] = wyt.transpose(1, 0, 2)
            else:
                wgt2[do, :, :, 768:1024] = wyt.transpose(1, 0, 2)

        srcv = imgs_src[s].astype(np.float32).reshape(2, 128, 256, 3)
        src = srcv.transpose(1, 3, 0, 2)  # (128, 3, 2, 256)

        in_maps.append({
            "pm": np.ascontiguousarray(pm, np.float32),
            "ca": np.ascontiguousarray(ca).astype(bfh),
            "wgt1": np.ascontiguousarray(wgt1).astype(bfh),
            "wgt2": np.ascontiguousarray(wgt2).astype(bfh),
            "src": np.ascontiguousarray(src).astype(bfh),
        })
    return in_maps, True


def kernel(colors, alphas, imgs_src, mpi_planes, pose_tgt, intrins_src,
           intrins_tgt):
    colors = np.asarray(colors, np.float32)
    alphas = np.asarray(alphas, np.float32)
    imgs_src = np.asarray(imgs_src, np.float32)
    mpi_planes = np.asarray(mpi_planes, np.float32)
    pose_tgt = np.asarray(pose_tgt, np.float32)
    intrins_src = np.asarray(intrins_src, np.float32)
    intrins_tgt = np.asarray(intrins_tgt, np.float32)

    in_maps, separable = _host_prepare(
        colors, alphas, imgs_src, mpi_planes, pose_tgt, intrins_src,
        intrins_tgt)
    if not separable:
        return _reference_numpy(colors, alphas, imgs_src, mpi_planes,
                                pose_tgt, intrins_src, intrins_tgt)

    from concourse.bass_utils import run_bass_kernel_spmd

    nc = _build_bass_program()
    try:
        res = run_bass_kernel_spmd(nc, in_maps, core_ids=list(range(NCORES)))
    except ModuleNotFoundError:
        # Containers without antenv.axon_hooks crash in the BASS_TRACE
        # path before executing; rerun untraced.
        os.environ["BASS_NEVER_TRACE"] = "1"
        res = run_bass_kernel_spmd(nc, in_maps, core_ids=list(range(NCORES)))
    _CACHED["last_results"] = res

    out = np.empty((P, S, NCH, H, W), np.float32)
    for core in range(NCORES):
        s, ph = divmod(core, 2)
        r = res.results[core]
        # device store orders: out1 = [T, srcRGB], out2a = acc, out2b = bro
        o1 = np.asarray(r["out1"]).astype(np.float32).reshape(
            POWN, 128, 4, 2, 256)
        o2a = np.asarray(r["out2a"]).astype(np.float32).reshape(
            POWN, 128, 3, 2, 256)
        o2b = np.asarray(r["out2b"]).astype(np.float32).reshape(
            POWN, 128, 3, 2, 256)
        stacked = np.concatenate(
            [o1[:, :, 0:1], o2a, o2b, o1[:, :, 1:4]], axis=2)
        # [do, p, ci, h, xo] -> out[d, s, ci, h*128+p, xo]
        out[ph * POWN:(ph + 1) * POWN, s] = (
            stacked.transpose(0, 2, 3, 1, 4).reshape(POWN, NCH, H, W))
    return out

